# revision 5
# baseline (speedup 1.0000x reference)
"""ChebyNet (K=4, 2-layer ChebConv + log_softmax) on 8 Trainium2 NeuronCores.

Strategy (1D node-parallel, per sharding hint):
  - Nodes are split contiguously across 8 cores, then within a core sorted by
    in-degree (descending). 128-node blocks are grouped into ~10 gather groups
    with a uniform slot count D per group, so each propagation is ~10 batched
    indirect DMAs (one per group) + ~10 segmented vector reduces (4D AP).
  - The symmetric normalization is folded into node features:
        prop(v) = -dis .* segsum(u[src]),   u = dis .* v
    u0 = dis .* x is precomputed on the host (degrees are host-computed
    structural data, as in the baseline); later u tables are exchanged with a
    Shared-output AllGather per propagation.
  - All K Chebyshev terms live in one [P, J, K*F] "txcat" tile; the dense
    projection sum_k Tx_k @ W_k is ONE PE transpose + copy + matmul per
    128-node block (contraction over K*F = 128), using host-packed Wcat.
  - ELL padding slots point at a guaranteed-zero pad row (rank >= OWN of
    core 0), so no zero-region maintenance is needed.

kernel(**inputs) takes the FULL inputs and returns the FULL [N, C] output.
"""

import os
import sys

import numpy as np

for _p in ("/opt/trn_rl_repo",):
    if os.path.isdir(_p) and _p not in sys.path:
        sys.path.insert(0, _p)

from contextlib import ExitStack

import concourse.bacc as bacc
import concourse.mybir as mybir
import concourse.tile as tile
from concourse.bass import AP, IndirectOffsetOnAxis
from concourse.bass_utils import run_bass_kernel_spmd
from concourse.masks import make_identity

P = 128
NCORES = 8
F32 = mybir.dt.float32
I32 = mybir.dt.int32
ALU = mybir.AluOpType
AF = mybir.ActivationFunctionType
AX = mybir.AxisListType

GCAP = 192      # max slots per gather group (L1: 192*32*4B = 24KB/partition)
DWASTE = 8      # max slot padding per block within a group
# Shared-space AllGather output measured faster end-to-end than Local
# (0.91s vs 1.18s rep-slope): Local delivers 8x the bytes per core and the
# collectives contend with the qPoolDynamic gather stream either way.
SHARED_AG = True


# ---------------------------------------------------------------------------
# host-side graph partitioning / grouped-ELL construction
# ---------------------------------------------------------------------------

def preprocess(x, edge_index, W1, b1, W2, b2):
    N, F_IN = x.shape
    K, _, HID = W1.shape
    C_OUT = W2.shape[2]
    src = np.asarray(edge_index[0], dtype=np.int64)
    dst = np.asarray(edge_index[1], dtype=np.int64)
    E = src.shape[0]

    OWN = N // NCORES
    assert OWN * NCORES == N
    # always keep at least one padding rank per core (zero row for ELL pads)
    J = OWN // P + 1 if OWN % P == 0 else (OWN + P - 1) // P
    RPAD = P * J
    TROWS = NCORES * RPAD
    ZIDX = RPAD - 1          # last row of core 0's segment: always a pad rank

    deg = np.bincount(dst, minlength=N).astype(np.int64)

    # per-core degree sort: rank 0 = highest degree
    rank_of = np.empty(N, np.int64)
    deg_rank = np.zeros((NCORES, RPAD), np.int64)
    for c in range(NCORES):
        dc = deg[c * OWN:(c + 1) * OWN]
        order = np.argsort(-dc, kind="stable")
        rank_of[c * OWN + order] = np.arange(OWN)
        deg_rank[c, :OWN] = dc[order]

    node_core = np.arange(N) // OWN
    p_of = rank_of % P
    j_of = rank_of // P
    urow_of = node_core * RPAD + p_of * J + j_of

    # per-block slot count: max degree in block over all cores (shared NEFF)
    degblk = deg_rank.reshape(NCORES, J, P)
    Dlist = np.maximum(degblk.max(axis=(0, 2)), 1).astype(np.int64)

    OFF = np.concatenate([[0], np.cumsum(Dlist)]).astype(np.int64)
    SUMDP = int(OFF[-1])
    colbase = OFF[:J]

    # runs of consecutive blocks with equal D (one 4D-AP reduce per run)
    RCAP = 96  # max slots per run: [128, 96, 32] f32 = 12KB/partition
    runs = []  # (j0, nb, D)
    j = 0
    while j < J:
        D = int(Dlist[j])
        nb = 1
        while (j + nb < J and int(Dlist[j + nb]) == D
               and (nb + 1) * D <= RCAP):
            nb += 1
        runs.append((j, nb, D))
        j += nb

    # edge -> (core, rank, slot) -> idx_tab column
    e_c = dst // OWN
    e_r = rank_of[dst]
    order_e = np.lexsort((urow_of[src], e_r, e_c))
    es = src[order_e]
    ec = e_c[order_e]
    er = e_r[order_e]
    gid = ec * RPAD + er
    counts = np.bincount(gid, minlength=NCORES * RPAD)
    starts = np.concatenate([[0], np.cumsum(counts)[:-1]])
    slot = np.arange(E) - starts[gid]
    ep = er % P
    ej = er // P
    col = colbase[ej] + slot
    idx_tab = np.full((NCORES, P, SUMDP), ZIDX, np.int32)
    idx_tab[ec, ep, col] = urow_of[es].astype(np.int32)

    # float-side host prep: dis scaling, permuted tables, packed weights
    dis = np.where(deg > 0, 1.0 / np.sqrt(np.maximum(deg, 1.0)), 0.0)
    x_perm = np.zeros((TROWS, F_IN), np.float32)
    x_perm[urow_of] = np.asarray(x, np.float32)
    u0_perm = np.zeros((TROWS, F_IN), np.float32)
    u0_perm[urow_of] = (dis[:, None] * np.asarray(x, np.float64)).astype(
        np.float32)
    dis_full = np.zeros((TROWS,), np.float32)
    dis_full[urow_of] = dis.astype(np.float32)
    dis_grid = dis_full.reshape(NCORES, P, J)

    S = 16
    KF = K * F_IN
    assert KF <= 128 and HID <= S and C_OUT <= S
    wc1 = np.zeros((KF, S), np.float32)
    for k in range(K):
        wc1[k * F_IN:(k + 1) * F_IN, :HID] = np.asarray(W1[k], np.float32)
    wc2 = np.zeros((KF, S), np.float32)
    for k in range(K):
        wc2[k * F_IN:k * F_IN + HID, :C_OUT] = np.asarray(W2[k], np.float32)
    b1r = np.broadcast_to(np.asarray(b1, np.float32), (P, HID)).copy()
    b2r = np.broadcast_to(np.asarray(b2, np.float32), (P, C_OUT)).copy()

    cfg = dict(
        N=N, E=E, F_IN=F_IN, HID=HID, C_OUT=C_OUT, K=K, KF=KF, S=S,
        OWN=OWN, J=J, RPAD=RPAD, TROWS=TROWS, SUMDP=SUMDP,
        Dlist=[int(d) for d in Dlist], OFF=[int(o) for o in OFF],
        runs=[(int(a), int(b), int(c)) for a, b, c in runs],
    )
    in_maps = []
    for c in range(NCORES):
        in_maps.append({
            "u0_perm": u0_perm,
            "xown": np.ascontiguousarray(x_perm[c * RPAD:(c + 1) * RPAD]),
            "dis_own": np.ascontiguousarray(dis_grid[c]),
            "idx_tab": np.ascontiguousarray(idx_tab[c]),
            "wc1": wc1,
            "wc2": wc2,
            "b1r": b1r,
            "b2r": b2r,
        })
    return cfg, in_maps, urow_of


# ---------------------------------------------------------------------------
# AP helpers
# ---------------------------------------------------------------------------

def _bcast_last(ap, n):
    """[P, c] -> [P, c, n] (innermost broadcast)."""
    return AP(ap.tensor, ap.offset, [*ap.ap, [0, n]])


def _bcast_mid(ap, reps):
    """[P, n] -> [P, reps, n] (middle broadcast)."""
    return AP(ap.tensor, ap.offset, [ap.ap[0], [0, reps], *ap.ap[1:]])


def _regroup(ap, nb, DH, F):
    """[P, nb*DH, F] gather view -> [P, nb, F, DH] (reduce innermost DH)."""
    assert ap.ap[1][1] == nb * DH and ap.ap[2][1] == F
    fstride = ap.ap[2][0]
    sstride = ap.ap[1][0]
    return AP(ap.tensor, ap.offset,
              [ap.ap[0], [sstride * DH, nb], [fstride, F], [sstride, DH]])


# ---------------------------------------------------------------------------
# device program
# ---------------------------------------------------------------------------

def build(cfg, rep=1):
    J = cfg["J"]
    RPAD = cfg["RPAD"]
    TROWS = cfg["TROWS"]
    F_IN = cfg["F_IN"]
    HID = cfg["HID"]
    C_OUT = cfg["C_OUT"]
    K = cfg["K"]
    KF = cfg["KF"]
    S = cfg["S"]
    SUMDP = cfg["SUMDP"]
    Dlist = cfg["Dlist"]
    OFF = cfg["OFF"]
    runs = cfg["runs"]

    nc = bacc.Bacc(
        "TRN2", target_bir_lowering=False, debug=False,
        enable_asserts=False, num_devices=NCORES,
    )

    u0_in = nc.dram_tensor("u0_perm", [TROWS, F_IN], F32, kind="ExternalInput")
    xown_in = nc.dram_tensor("xown", [RPAD, F_IN], F32, kind="ExternalInput")
    dis_in = nc.dram_tensor("dis_own", [P, J], F32, kind="ExternalInput")
    idx_in = nc.dram_tensor("idx_tab", [P, SUMDP], I32, kind="ExternalInput")
    wc1_in = nc.dram_tensor("wc1", [KF, S], F32, kind="ExternalInput")
    wc2_in = nc.dram_tensor("wc2", [KF, S], F32, kind="ExternalInput")
    b1_in = nc.dram_tensor("b1r", [P, HID], F32, kind="ExternalInput")
    b2_in = nc.dram_tensor("b2r", [P, C_OUT], F32, kind="ExternalInput")
    y_out = nc.dram_tensor("y", [RPAD, C_OUT], F32, kind="ExternalOutput")

    rg = [list(range(NCORES))]
    ag_space = "Shared" if SHARED_AG else "Local"

    with ExitStack() as ctx:
        tc = ctx.enter_context(tile.TileContext(nc))
        dram = ctx.enter_context(tc.tile_pool(name="dram", bufs=1, space="DRAM"))
        cpool = ctx.enter_context(tc.tile_pool(name="const", bufs=1))

        ident = cpool.tile([P, P], F32)
        make_identity(nc, ident)
        idx_sb = cpool.tile([P, SUMDP], I32)
        nc.sync.dma_start(out=idx_sb, in_=idx_in.ap())
        wc1_sb = cpool.tile([KF, S], F32)
        nc.sync.dma_start(out=wc1_sb, in_=wc1_in.ap())
        wc2_sb = cpool.tile([KF, S], F32)
        nc.sync.dma_start(out=wc2_sb, in_=wc2_in.ap())
        b1_sb = cpool.tile([P, HID], F32)
        nc.sync.dma_start(out=b1_sb, in_=b1_in.ap())
        b2_sb = cpool.tile([P, C_OUT], F32)
        nc.sync.dma_start(out=b2_sb, in_=b2_in.ap())
        dis_sb = cpool.tile([P, J], F32)
        nc.sync.dma_start(out=dis_sb, in_=dis_in.ap())
        ndis_sb = cpool.tile([P, J], F32)
        nc.vector.tensor_scalar(ndis_sb, dis_sb, -1.0, None, ALU.mult)
        m2dis_sb = cpool.tile([P, J], F32)
        nc.vector.tensor_scalar(m2dis_sb, dis_sb, -2.0, None, ALU.mult)

        work = ctx.enter_context(tc.tile_pool(name="work", bufs=1))
        zp = ctx.enter_context(tc.tile_pool(name="zp", bufs=3))
        gp = ctx.enter_context(tc.tile_pool(name="gp", bufs=4))
        tsp = ctx.enter_context(tc.tile_pool(name="tsp", bufs=2))
        pp = ctx.enter_context(tc.tile_pool(name="pp", bufs=2, space="PSUM"))
        ap_ = ctx.enter_context(tc.tile_pool(name="acc", bufs=1, space="PSUM"))

        yv = y_out.ap().rearrange("(p j) c -> p j c", p=P)

        def body(r):
            txcat = cpool.tile([P, J, KF], F32, tag="txcat", name=f"txcat{r}")

            def gather_props(u_first, F, lname):
                """Run props k=1..K-1 for one layer; writes txcat slices.

                Yields after each tx_k is written so the caller can emit the
                u-table store/allgather for it.
                """
                ufull = u_first
                for k in range(1, K):
                    z = zp.tile([P, J, F_IN], F32, tag="z",
                                name=f"{lname}z{k}_{r}")
                    for (j0, nb, Dj) in runs:
                        ns = nb * Dj
                        g = gp.tile([P, 96, F_IN], F32, tag="g",
                                    name=f"{lname}g{k}_{j0}_{r}")
                        for c in range(ns):
                            col = OFF[j0] + c
                            nc.gpsimd.indirect_dma_start(
                                out=g[:, c, :F],
                                out_offset=None,
                                in_=ufull,
                                in_offset=IndirectOffsetOnAxis(
                                    ap=idx_sb[:, col:col + 1], axis=0),
                            )
                        nc.vector.tensor_reduce(
                            out=z[:, j0:j0 + nb, :F],
                            in_=_regroup(g[:, :ns, :F], nb, Dj, F),
                            axis=AX.X, op=ALU.add)
                    txk = AP(txcat.tensor, txcat.offset + k * F_IN,
                             [txcat.ap[0], [KF, J], [1, F]])
                    if k == 1:
                        nc.vector.tensor_tensor(
                            out=txk, in0=z[:, :, :F],
                            in1=_bcast_last(ndis_sb, F), op=ALU.mult)
                    else:
                        zt = zp.tile([P, J, F_IN], F32, tag="z",
                                     name=f"{lname}zt{k}_{r}")
                        nc.vector.tensor_tensor(
                            out=zt[:, :, :F], in0=z[:, :, :F],
                            in1=_bcast_last(m2dis_sb, F), op=ALU.mult)
                        txprev = AP(txcat.tensor,
                                    txcat.offset + (k - 2) * F_IN,
                                    [txcat.ap[0], [KF, J], [1, F]])
                        nc.vector.tensor_tensor(
                            out=txk, in0=zt[:, :, :F], in1=txprev,
                            op=ALU.subtract)
                    if k < K - 1:
                        un = work.tile([P, J, F_IN], F32, tag="un",
                                       bufs=2, name=f"{lname}un{k}_{r}")
                        nc.vector.tensor_tensor(
                            out=un[:, :, :F], in0=txk,
                            in1=_bcast_last(dis_sb, F), op=ALU.mult)
                        uo = dram.tile([RPAD, F], F32, tag=f"uo{lname}{k}",
                                       name=f"{lname}uo{k}_{r}")
                        nc.sync.dma_start(
                            out=uo.rearrange("(p j) f -> p j f", p=P),
                            in_=un[:, :, :F])
                        uf = dram.tile([TROWS, F], F32,
                                       tag=f"uf{lname}{k}_{r}",
                                       name=f"{lname}uf{k}_{r}",
                                       addr_space=ag_space)
                        nc.gpsimd.collective_compute(
                            "AllGather", ALU.bypass, replica_groups=rg,
                            ins=[uo.opt()], outs=[uf.opt()])
                        # 128B-granularity indirect reads from collective-
                        # output buffers are ~3x slower than from plain DRAM;
                        # one bulk copy (affine DMA) makes the 2000 random
                        # gather reads per prop run at normal speed.
                        ufc = dram.tile([TROWS, F], F32,
                                        tag=f"ufc{lname}{k}_{r}",
                                        name=f"{lname}ufc{k}_{r}")
                        nc.sync.dma_start(out=ufc, in_=uf)
                        ufull = ufc

            def proj(w_sb, pname):
                acc = ap_.tile([P, J * S], F32, space="PSUM", tag="acc",
                               bufs=1, name=f"{pname}acc_{r}")
                for j in range(J):
                    tp = pp.tile([P, P], F32, space="PSUM", tag="tp",
                                 name=f"{pname}tp{j}_{r}")
                    nc.tensor.transpose(out=tp, in_=txcat[:, j, :],
                                        identity=ident)
                    ts = tsp.tile([P, P], F32, tag="ts",
                                  name=f"{pname}ts{j}_{r}")
                    nc.scalar.copy(out=ts, in_=tp)
                    nc.tensor.matmul(
                        out=acc[:, j * S:(j + 1) * S],
                        lhsT=ts, rhs=w_sb, start=True, stop=True)
                return acc.rearrange("p (j s) -> p j s", s=S)

            # ---- layer 1 ----------------------------------------------
            nc.sync.dma_start(
                out=AP(txcat.tensor, txcat.offset,
                       [txcat.ap[0], [KF, J], [1, F_IN]]),
                in_=xown_in.ap().rearrange("(p j) f -> p j f", p=P))
            gather_props(u0_in.ap(), F_IN, f"l1")
            acc1 = proj(wc1_sb, "p1")
            h1 = work.tile([P, J, HID], F32, tag="h1", name=f"h1_{r}")
            nc.vector.tensor_add(h1, acc1[:, :, :HID], _bcast_mid(b1_sb, J))
            nc.scalar.activation(h1, h1, AF.Relu)

            # ---- layer 2 ----------------------------------------------
            nc.vector.memset(txcat, 0.0)
            nc.vector.tensor_copy(
                out=AP(txcat.tensor, txcat.offset,
                       [txcat.ap[0], [KF, J], [1, HID]]),
                in_=h1)
            un0 = work.tile([P, J, HID], F32, tag="un0", name=f"l2un0_{r}")
            nc.vector.tensor_mul(un0, h1, _bcast_last(dis_sb, HID))
            uo0 = dram.tile([RPAD, HID], F32, tag="uol2_0",
                            name=f"l2uo0_{r}")
            nc.sync.dma_start(
                out=uo0.rearrange("(p j) f -> p j f", p=P), in_=un0)
            uf0 = dram.tile([TROWS, HID], F32, tag=f"ufl2_0_{r}",
                            name=f"l2uf0_{r}", addr_space=ag_space)
            nc.gpsimd.collective_compute(
                "AllGather", ALU.bypass, replica_groups=rg,
                ins=[uo0.opt()], outs=[uf0.opt()])
            uf0c = dram.tile([TROWS, HID], F32, tag=f"ufc_l2_0_{r}",
                             name=f"l2ufc0_{r}")
            nc.sync.dma_start(out=uf0c, in_=uf0)
            gather_props(uf0c, HID, f"l2")
            acc2 = proj(wc2_sb, "p2")

            # ---- log_softmax ------------------------------------------
            s2 = work.tile([P, J, C_OUT], F32, tag="s2", name=f"s2_{r}")
            nc.vector.tensor_add(s2, acc2[:, :, :C_OUT], _bcast_mid(b2_sb, J))
            mx = work.tile([P, J], F32, tag="mx", name=f"mx_{r}")
            nc.vector.tensor_reduce(out=mx, in_=s2, axis=AX.X, op=ALU.max)
            sh = work.tile([P, J, C_OUT], F32, tag="sh", name=f"sh_{r}")
            nc.vector.tensor_tensor(
                out=sh, in0=s2, in1=_bcast_last(mx, C_OUT), op=ALU.subtract)
            ex = work.tile([P, J, C_OUT], F32, tag="ex", name=f"ex_{r}")
            nc.scalar.activation(ex, sh, AF.Exp)
            ssum = work.tile([P, J], F32, tag="ssum", name=f"ssum_{r}")
            nc.vector.tensor_reduce(out=ssum, in_=ex, axis=AX.X, op=ALU.add)
            lg = work.tile([P, J], F32, tag="lg", name=f"lg_{r}")
            nc.scalar.activation(lg, ssum, AF.Ln)
            yt = work.tile([P, J, C_OUT], F32, tag="ex", name=f"yt_{r}")
            nc.vector.tensor_tensor(
                out=yt, in0=sh, in1=_bcast_last(lg, C_OUT), op=ALU.subtract)
            nc.sync.dma_start(out=yv, in_=yt)

        for r in range(rep):
            body(r)

    nc.compile()
    return nc


# ---------------------------------------------------------------------------
# entry point
# ---------------------------------------------------------------------------

_LAST_PERF = {}


def kernel(x, edge_index, W1, b1, W2, b2):
    cfg, in_maps, urow_of = preprocess(x, edge_index, W1, b1, W2, b2)
    nc = build(cfg)
    trace = bool(int(os.environ.get("GNN_TRACE", "0")))
    res = run_bass_kernel_spmd(
        nc, in_maps, core_ids=list(range(NCORES)), trace=trace)
    _LAST_PERF.clear()
    _LAST_PERF.update(
        exec_time_ns=res.exec_time_ns,
        mean_exec_time_ns=res.mean_exec_time_ns,
        trace=res.instructions_and_trace[1] if res.instructions_and_trace else None,
    )
    full_y = np.concatenate([res.results[c]["y"] for c in range(NCORES)], axis=0)
    return np.ascontiguousarray(full_y[urow_of]).astype(np.float32)

